# revision 25
# baseline (speedup 1.0000x reference)
"""Trainium2 Bass kernel for nn_AnalyticalStage2 (v5).

Math (per batch row b, time index i, constant per-row decay d):
    v_i = d*v_{i-1} + p_i,   omega_i = A*p_i + c*v_{i-1},  c = D*(1-d)

Pair reformulation (halves the serial DVE scan):
    w_k := v_{2k+1} satisfies  w_k = d^2 * w_{k-1} + u_k,
    u_k  = d*pe_k + po_k          (pe=p_even, po=p_odd)
    om_e_k = A*pe_k + c*w_{k-1}
    om_o_k = A*po_k + c*d*w_{k-1} + c*pe_k

Mapping: 512 rows -> 8 cores x 64 rows. Partitions = 2 time halves x 64
rows (q = h*64 + b); per-partition sequence = 8192 pairs. Host stages p
as bf16 deinterleaved [q, parity*8192 + k]; output staged bf16
TILE-INTERLEAVED (x = 2048*t + 1024*e + j), re-interleaved + upcast on
host.

Engine split per W=1024 tile:
  - GpSimd: u = d*pe + po (scalar_tensor_tensor, SBUF bf16)
  - DVE:    tensor_tensor_scan w (fp32 state) into per-tile w tiles
            (pool bufs=4 -- avoids the tile-granular WAR that a single
            persistent w buffer creates between scan(t+1) and
            combine(t)'s PE reads)
  - PE:     combine into double-buffered merged [128,2048] PSUM
            (stationary-batched: A x4, c x4, cd x2)
  - ACT:    w boundary copy + ONE merged drain per tile
Inputs ride HWDGE/Q1 (nc.sync), outputs SWDGE/Q0 (nc.gpsimd). ombuf is
split in two tiles (tiles 0-3 / 4-7) so the half-1 stream-out DMA never
WAR-blocks later drains.

Half 2 scans from 0; tail fixup om2 += q_c * G2[x] with G2 the
[128,4096] tile-interleaved geometric table (d folded into odd blocks)
and per-chunk scalars q_c = c*v1e*(dd^2048)^c: DVE tensor_scalar (4x) +
tensor_add (2x) in 4 chunks overlapped with out-DMAs.
"""

import numpy as np
import ml_dtypes

import concourse.bass as bass
import concourse.bacc as bacc
import concourse.mybir as mybir
from concourse.bass_utils import run_bass_kernel_spmd
from concourse.tile import TileContext

_C = 0.206756
B, NT = 512, 32768
NCORES = 8
BLOC = B // NCORES  # 64
DELTA = 0.2 / (NT - 1)

F32 = mybir.dt.float32
BF16 = mybir.dt.bfloat16
ALU = mybir.AluOpType
ACTF = mybir.ActivationFunctionType

TH = NT // 2  # half length 16384
NK = TH // 2  # pairs per half 8192
W = 1024  # compute tile width (pairs)
NTILES = NK // W  # 8
MM = 512  # matmul free-dim chunk (one PSUM bank)

# input DMA chunks per parity: (lo, width) in pairs
IN_CHUNKS = [(0, 1024), (1024, 3072), (4096, 4096)]

BF = ml_dtypes.bfloat16


def build(nc):
    p_ext = nc.declare_dram_parameter("p", [128, 2 * NK], BF16, isOutput=False)
    hr_ext = nc.declare_dram_parameter("h_raw", [128, 160], F32, isOutput=False)
    out_ext = nc.declare_dram_parameter("out", [128, 2 * NK], BF16, isOutput=True)

    with TileContext(nc) as tc:
        with (
            tc.tile_pool(name="const", bufs=1) as cpool,
            tc.tile_pool(name="big", bufs=1) as bigpool,
            tc.tile_pool(name="pb", bufs=2) as bpool,
            tc.tile_pool(name="w", bufs=4) as wpool,
            tc.tile_pool(name="fx", bufs=3) as fxpool,
            tc.tile_pool(name="st", bufs=4) as stpool,
            tc.tile_pool(name="psu", bufs=2, space="PSUM") as psu,
            tc.tile_pool(name="pse", bufs=1, space="PSUM") as pse,
            tc.tile_pool(name="pso", bufs=1, space="PSUM") as pso,
        ):
            # ---- input DMAs: params+identity first, then p, all on Q1 ----
            hr = cpool.tile([128, 160], F32)
            nc.sync.dma_start(out=hr[:, :], in_=hr_ext[:])

            pch = []  # [(e, lo, width, tile), ...]
            for lo, wd in IN_CHUNKS:
                for e in range(2):
                    t = bpool.tile([128, wd], BF16, tag=f"pb{wd}")
                    nc.sync.dma_start(
                        out=t[:, :], in_=p_ext[:, e * NK + lo : e * NK + lo + wd]
                    )
                    pch.append((e, lo, wd, t))

            def pslice(e, lo, width):
                for pe_, clo, cw, tl in pch:
                    if pe_ == e and clo <= lo and lo + width <= clo + cw:
                        return tl[:, lo - clo : lo - clo + width]
                raise AssertionError((e, lo, width))

            # ---- params on all 128 partitions ----
            E1, E2, eta = hr[:, 0:1], hr[:, 1:2], hr[:, 2:3]
            I01 = hr[:, 32:160]  # host-supplied 0/1 identity
            prm = cpool.tile([128, 16], F32)

            def pc(i):
                return prm[:, i : i + 1]

            s, se, rse, e12 = pc(0), pc(1), pc(2), pc(3)
            alpha, lnd, d, rs = pc(4), pc(5), pc(6), pc(7)
            A, rE2, t2, t3 = pc(8), pc(9), pc(10), pc(11)
            D, omd, c, dd = pc(12), pc(13), pc(14), pc(15)

            nc.vector.tensor_add(out=s, in0=E1, in1=E2)
            nc.vector.tensor_mul(out=se, in0=s, in1=eta)
            nc.vector.reciprocal(rse, se)
            nc.vector.tensor_mul(out=e12, in0=E1, in1=E2)
            nc.vector.tensor_mul(out=alpha, in0=e12, in1=rse)
            nc.vector.tensor_scalar_mul(lnd, alpha, -DELTA)
            nc.scalar.activation(d, lnd, ACTF.Exp)
            nc.vector.reciprocal(rs, s)
            nc.vector.tensor_scalar_mul(A, rs, _C)
            nc.vector.reciprocal(rE2, E2)
            nc.vector.tensor_mul(out=t2, in0=E1, in1=rE2)
            nc.vector.tensor_mul(out=t3, in0=t2, in1=rs)
            nc.vector.tensor_scalar_mul(D, t3, _C)
            nc.vector.tensor_scalar(omd, d, -1.0, 1.0, ALU.mult, ALU.add)
            nc.vector.tensor_mul(out=c, in0=D, in1=omd)
            nc.vector.tensor_mul(out=dd, in0=d, in1=d)

            prm2 = cpool.tile([128, 8], F32)
            cd = prm2[:, 0:1]
            lndd = prm2[:, 1:2]
            dk3 = prm2[:, 2:3]
            nc.vector.tensor_mul(out=cd, in0=c, in1=d)

            # diag stationaries first -- they gate the PE pipeline start
            diag_d = cpool.tile([128, 128], BF16)
            diag_A = cpool.tile([128, 128], BF16)
            diag_c = cpool.tile([128, 128], BF16)
            diag_cd = cpool.tile([128, 128], BF16)
            nc.vector.tensor_scalar_mul(diag_d[:], I01, d)
            nc.vector.tensor_scalar_mul(diag_A[:], I01, A)
            nc.vector.tensor_scalar_mul(diag_c[:], I01, c)
            nc.vector.tensor_scalar_mul(diag_cd[:], I01, cd)

            dks2 = cpool.tile([128, 13], F32)

            def emit_dks2():
                nc.vector.tensor_scalar_mul(lndd, lnd, 2.0)
                nc.scalar.copy(out=dks2[:, 0:1], in_=dd)
                for j in range(1, 13):
                    nc.vector.tensor_mul(
                        out=dks2[:, j : j + 1],
                        in0=dks2[:, j - 1 : j],
                        in1=dks2[:, j - 1 : j],
                    )
                nc.vector.tensor_mul(out=dk3, in0=dks2[:, 11:12], in1=dks2[:, 12:13])

            # ---- G2: tile-interleaved geometric table over tiles 0-1:
            # G2[:, 1024e + j] = d^e * dd^j;  [2048:4096] = that * dd^1024 ----
            ramp = cpool.tile([128, 1024], F32)
            nc.gpsimd.iota(
                out=ramp[:],
                pattern=[[1, 1024]],
                base=0,
                channel_multiplier=0,
                allow_small_or_imprecise_dtypes=True,
            )
            G2 = bigpool.tile([128, 4096], BF16)
            g2_steps = [
                lambda: nc.scalar.activation(
                    G2[:, 0:1024], ramp[:], ACTF.Exp, scale=lndd
                ),
                lambda: nc.scalar.activation(
                    G2[:, 1024:2048], G2[:, 0:1024], ACTF.Copy, scale=d
                ),
                lambda: nc.scalar.activation(
                    G2[:, 2048:4096], G2[:, 0:2048], ACTF.Copy, scale=dks2[:, 10:11]
                ),
            ]

            # tile-interleaved output buffers: A = tiles 0-3, B = tiles 4-7
            ombufA = bigpool.tile([128, NK], BF16)
            ombufB = bigpool.tile([128, NK], BF16)

            def ombuf(t):
                return (ombufA, 2 * W * t) if t < 4 else (ombufB, 2 * W * (t - 4))

            zcol = cpool.tile([128, 1], BF16)
            nc.vector.memset(zcol[:, :], 0.0)

            identb = cpool.tile([128, 128], BF16)
            nc.scalar.copy(out=identb[:, :], in_=I01)

            def junk_mms(ut, pe, n):
                # HAM filler: overwritten by the next start=True matmul into
                # the same bank; keeps the PE activity window busy.
                for i in range(n):
                    nc.tensor.matmul(
                        ut[:, (i % 2) * MM : (i % 2) * MM + MM],
                        diag_A[:],
                        pe[:, 0:MM],
                        start=True,
                        stop=True,
                        skip_group_check=True,
                    )

            def u_tile(t, warm=0):
                lo = t * W
                pe = pslice(0, lo, W)
                po = pslice(1, lo, W)
                ut = psu.tile([128, W], F32, tag="u")
                junk_mms(ut, pe, warm)
                for q in range(W // MM):
                    nc.tensor.matmul(
                        ut[:, q * MM : (q + 1) * MM],
                        diag_d[:],
                        pe[:, q * MM : (q + 1) * MM],
                        start=True,
                        stop=False,
                    )
                for q in range(W // MM):
                    nc.tensor.matmul(
                        ut[:, q * MM : (q + 1) * MM],
                        identb[:],
                        po[:, q * MM : (q + 1) * MM],
                        start=False,
                        stop=True,
                    )
                return ut

            u_tiles = {0: u_tile(0, warm=2)}

            # ---- main loop ----
            prev_w = None
            for t in range(NTILES):
                lo = t * W
                ups = u_tiles.pop(t)
                if t + 1 < NTILES:
                    u_tiles[t + 1] = u_tile(t + 1)

                wt = wpool.tile([128, W + 1], BF16, tag="w")
                init = zcol[:, 0:1] if prev_w is None else prev_w[:, W : W + 1]
                nc.vector.tensor_copy(wt[:, 0:1], init)
                nc.vector.tensor_tensor_scan(
                    out=wt[:, 1 : W + 1],
                    data0=dd.broadcast_to([128, W]),
                    data1=ups[:],
                    initial=init,
                    op0=ALU.mult,
                    op1=ALU.add,
                )

                pe = pslice(0, lo, W)
                po = pslice(1, lo, W)
                ome = pse.tile([128, W], F32, tag="ome")
                omo = pso.tile([128, W], F32, tag="omo")
                # e-group first (A, c) so its drain fires mid-combine
                for q in range(W // MM):
                    nc.tensor.matmul(
                        ome[:, q * MM : (q + 1) * MM],
                        diag_A[:],
                        pe[:, q * MM : (q + 1) * MM],
                        start=True,
                        stop=False,
                    )
                for q in range(W // MM):
                    nc.tensor.matmul(
                        ome[:, q * MM : (q + 1) * MM],
                        diag_c[:],
                        wt[:, q * MM : q * MM + MM],
                        start=False,
                        stop=True,
                    )
                for q in range(W // MM):
                    nc.tensor.matmul(
                        omo[:, q * MM : (q + 1) * MM],
                        diag_A[:],
                        po[:, q * MM : (q + 1) * MM],
                        start=True,
                        stop=False,
                    )
                for q in range(W // MM):
                    nc.tensor.matmul(
                        omo[:, q * MM : (q + 1) * MM],
                        diag_c[:],
                        pe[:, q * MM : (q + 1) * MM],
                        start=False,
                        stop=False,
                    )
                for q in range(W // MM):
                    nc.tensor.matmul(
                        omo[:, q * MM : (q + 1) * MM],
                        diag_cd[:],
                        wt[:, q * MM : q * MM + MM],
                        start=False,
                        stop=True,
                    )

                ob, og = ombuf(t)
                nc.scalar.copy(out=ob[:, og : og + W], in_=ome[:])
                nc.scalar.copy(out=ob[:, og + W : og + 2 * W], in_=omo[:])

                if t == 3:
                    nc.gpsimd.dma_start(
                        out=out_ext[0:64, 0:8192], in_=ombufA[0:64, :]
                    )
                elif t == 7:
                    nc.gpsimd.dma_start(
                        out=out_ext[0:64, 8192:16384], in_=ombufB[0:64, :]
                    )
                if t == 0:
                    emit_dks2()
                elif t in (1, 2, 3):
                    g2_steps[t - 1]()
                prev_w = wt

            # ---- tail: fix up half 2 (rows 64:128) ----
            v1e = cpool.tile([128, 1], BF16)
            nc.sync.dma_start(out=v1e[64:128, :], in_=prev_w[0:64, W : W + 1])
            qs = cpool.tile([128, 4], F32)
            nc.vector.tensor_mul(
                out=qs[64:128, 0:1], in0=prm[64:128, 14:15], in1=v1e[64:128, :]
            )
            for ci, dcol in ((1, dks2[64:128, 11:12]), (2, dks2[64:128, 12:13]),
                             (3, prm2[64:128, 2:3])):
                nc.vector.tensor_mul(
                    out=qs[64:128, ci : ci + 1], in0=qs[64:128, 0:1], in1=dcol
                )

            # 4 chunks of 4096 over the tile-interleaved x axis
            for ci in range(4):
                xlo = 4096 * ci
                ob = ombufA if ci < 2 else ombufB
                og = xlo if ci < 2 else xlo - 8192
                fix = fxpool.tile([128, 4096], BF16, tag="fix")
                stage = stpool.tile([128, 4096], BF16, tag="stage")
                if ci % 2 == 1:
                    # odd-chunk fix on ACT, fully parallel with DVE's chain
                    nc.scalar.activation(
                        fix[64:128, :],
                        G2[64:128, :],
                        ACTF.Copy,
                        scale=qs[64:128, ci : ci + 1],
                    )
                else:
                    nc.vector.tensor_scalar_mul(
                        fix[64:128, :], G2[64:128, :], qs[64:128, ci : ci + 1]
                    )
                nc.vector.tensor_add(
                    out=stage[64:128, :],
                    in0=fix[64:128, :],
                    in1=ob[64:128, og : og + 4096],
                )
                if ci < 3:
                    eng = nc.sync if ci % 2 == 0 else nc.gpsimd
                    eng.dma_start(
                        out=out_ext[64:128, xlo : xlo + 4096],
                        in_=stage[64:128, :],
                    )
                else:
                    # split the final chunk across both DMA queues
                    nc.sync.dma_start(
                        out=out_ext[64:128, xlo : xlo + 2048],
                        in_=stage[64:128, 0:2048],
                    )
                    nc.gpsimd.dma_start(
                        out=out_ext[64:128, xlo + 2048 : xlo + 4096],
                        in_=stage[64:128, 2048:4096],
                    )

    return nc


def make_nc():
    nc = bacc.Bacc(None)
    build(nc)
    nc.finalize()
    return nc


def _stage_p(p_core):
    # [64, 32768] f32 -> [128, 16384] bf16: q=h*64+b, x=e*8192+k
    x = np.asarray(p_core, dtype=BF).reshape(64, 2, NK, 2)
    return np.ascontiguousarray(x.transpose(1, 0, 3, 2).reshape(128, 2 * NK))


def _stage_hr(hr_core):
    # [64, 3] f32 -> [128, 160] f32: cols 0-2 params (rows duplicated
    # across halves), cols 32-159 a 0/1 identity matrix
    out = np.zeros((128, 160), dtype=np.float32)
    out[0:64, 0:3] = hr_core
    out[64:128, 0:3] = hr_core
    out[:, 32:160] = np.eye(128, dtype=np.float32)
    return out


def _unstage_out(o_core):
    # [128, 16384] bf16 tile-interleaved -> [64, 32768] f32
    x = np.asarray(o_core).reshape(2, 64, NTILES, 2, W)  # (h, b, t, e, j)
    x = x.transpose(1, 0, 2, 4, 3)  # (b, h, t, j, e)
    return np.ascontiguousarray(x.reshape(64, NT)).astype(np.float32)


def run(inputs, trace=False):
    nc = make_nc()
    p = np.asarray(inputs["p"], dtype=np.float32)
    hr = np.asarray(inputs["h_raw"], dtype=np.float32)
    in_maps = []
    for i in range(NCORES):
        sl = slice(i * BLOC, (i + 1) * BLOC)
        in_maps.append({"p": _stage_p(p[sl]), "h_raw": _stage_hr(hr[sl])})
    res = run_bass_kernel_spmd(nc, in_maps, core_ids=list(range(NCORES)), trace=trace)
    out = np.concatenate(
        [_unstage_out(res.results[i]["out"]) for i in range(NCORES)], axis=0
    )
    return out, res


def kernel(h, t, p, h_raw):
    out, _ = run({"p": p, "h_raw": h_raw})
    return out


# revision 27
# speedup vs baseline: 1.2332x; 1.2332x over previous
"""Trainium2 Bass kernel for nn_AnalyticalStage2 (v5).

Math (per batch row b, time index i, constant per-row decay d):
    v_i = d*v_{i-1} + p_i,   omega_i = A*p_i + c*v_{i-1},  c = D*(1-d)

Pair reformulation (halves the serial DVE scan):
    w_k := v_{2k+1} satisfies  w_k = d^2 * w_{k-1} + u_k,
    u_k  = d*pe_k + po_k          (pe=p_even, po=p_odd)
    om_e_k = A*pe_k + c*w_{k-1}
    om_o_k = A*po_k + c*d*w_{k-1} + c*pe_k

Mapping: 512 rows -> 8 cores x 64 rows. Partitions = 2 time halves x 64
rows (q = h*64 + b); per-partition sequence = 8192 pairs. Host stages p
as bf16 deinterleaved [q, parity*8192 + k]; output staged bf16
TILE-INTERLEAVED (x = 2048*t + 1024*e + j), re-interleaved + upcast on
host.

Engine split per W=1024 tile:
  - GpSimd: u = d*pe + po (scalar_tensor_tensor, SBUF bf16)
  - DVE:    tensor_tensor_scan w (fp32 state) into per-tile w tiles
            (pool bufs=4 -- avoids the tile-granular WAR that a single
            persistent w buffer creates between scan(t+1) and
            combine(t)'s PE reads)
  - PE:     combine into double-buffered merged [128,2048] PSUM
            (stationary-batched: A x4, c x4, cd x2)
  - ACT:    w boundary copy + ONE merged drain per tile
Inputs ride HWDGE/Q1 (nc.sync), outputs SWDGE/Q0 (nc.gpsimd). ombuf is
split in two tiles (tiles 0-3 / 4-7) so the half-1 stream-out DMA never
WAR-blocks later drains.

Half 2 scans from 0; tail fixup om2 += q_c * G2[x] with G2 the
[128,4096] tile-interleaved geometric table (d folded into odd blocks)
and per-chunk scalars q_c = c*v1e*(dd^2048)^c: DVE tensor_scalar (4x) +
tensor_add (2x) in 4 chunks overlapped with out-DMAs.
"""

import numpy as np
import ml_dtypes

import concourse.bass as bass
import concourse.bacc as bacc
import concourse.mybir as mybir
from concourse.bass_utils import run_bass_kernel_spmd
from concourse.tile import TileContext

_C = 0.206756
B, NT = 512, 32768
NCORES = 8
BLOC = B // NCORES  # 64
DELTA = 0.2 / (NT - 1)

F32 = mybir.dt.float32
BF16 = mybir.dt.bfloat16
ALU = mybir.AluOpType
ACTF = mybir.ActivationFunctionType

TH = NT // 2  # half length 16384
NK = TH // 2  # pairs per half 8192
W = 1024  # compute tile width (pairs)
NTILES = NK // W  # 8
MM = 512  # matmul free-dim chunk (one PSUM bank)

# input DMA chunks per parity: (lo, width) in pairs
IN_CHUNKS = [(0, 1024), (1024, 3072), (4096, 4096)]

BF = ml_dtypes.bfloat16


def build(nc):
    p_ext = nc.declare_dram_parameter("p", [128, 2 * NK], BF16, isOutput=False)
    hr_ext = nc.declare_dram_parameter("h_raw", [128, 160], F32, isOutput=False)
    out_ext = nc.declare_dram_parameter("out", [128, 2 * NK], BF16, isOutput=True)

    with TileContext(nc) as tc:
        with (
            tc.tile_pool(name="const", bufs=1) as cpool,
            tc.tile_pool(name="big", bufs=1) as bigpool,
            tc.tile_pool(name="pb", bufs=2) as bpool,
            tc.tile_pool(name="w", bufs=4) as wpool,
            tc.tile_pool(name="fx", bufs=2) as fxpool,
            tc.tile_pool(name="st", bufs=4) as stpool,
            tc.tile_pool(name="psu", bufs=2, space="PSUM") as psu,
            tc.tile_pool(name="pse", bufs=1, space="PSUM") as pse,
            tc.tile_pool(name="pso", bufs=1, space="PSUM") as pso,
        ):
            # ---- input DMAs: params+identity first, then p, all on Q1 ----
            hr = cpool.tile([128, 160], F32)
            nc.sync.dma_start(out=hr[:, :], in_=hr_ext[:])

            pch = []  # [(e, lo, width, tile), ...]
            for lo, wd in IN_CHUNKS:
                for e in range(2):
                    t = bpool.tile([128, wd], BF16, tag=f"pb{wd}")
                    nc.sync.dma_start(
                        out=t[:, :], in_=p_ext[:, e * NK + lo : e * NK + lo + wd]
                    )
                    pch.append((e, lo, wd, t))

            def pslice(e, lo, width):
                for pe_, clo, cw, tl in pch:
                    if pe_ == e and clo <= lo and lo + width <= clo + cw:
                        return tl[:, lo - clo : lo - clo + width]
                raise AssertionError((e, lo, width))

            # ---- params on all 128 partitions ----
            E1, E2, eta = hr[:, 0:1], hr[:, 1:2], hr[:, 2:3]
            I01 = hr[:, 32:160]  # host-supplied 0/1 identity
            prm = cpool.tile([128, 16], F32)

            def pc(i):
                return prm[:, i : i + 1]

            s, se, rse, e12 = pc(0), pc(1), pc(2), pc(3)
            alpha, lnd, d, rs = pc(4), pc(5), pc(6), pc(7)
            A, rE2, t2, t3 = pc(8), pc(9), pc(10), pc(11)
            D, omd, c, dd = pc(12), pc(13), pc(14), pc(15)

            nc.vector.tensor_add(out=s, in0=E1, in1=E2)
            nc.vector.tensor_mul(out=se, in0=s, in1=eta)
            nc.vector.reciprocal(rse, se)
            nc.vector.tensor_mul(out=e12, in0=E1, in1=E2)
            nc.vector.tensor_mul(out=alpha, in0=e12, in1=rse)
            nc.vector.tensor_scalar_mul(lnd, alpha, -DELTA)
            nc.scalar.activation(d, lnd, ACTF.Exp)
            nc.vector.reciprocal(rs, s)
            nc.vector.tensor_scalar_mul(A, rs, _C)
            nc.vector.reciprocal(rE2, E2)
            nc.vector.tensor_mul(out=t2, in0=E1, in1=rE2)
            nc.vector.tensor_mul(out=t3, in0=t2, in1=rs)
            nc.vector.tensor_scalar_mul(D, t3, _C)
            nc.vector.tensor_scalar(omd, d, -1.0, 1.0, ALU.mult, ALU.add)
            nc.vector.tensor_mul(out=c, in0=D, in1=omd)
            nc.vector.tensor_mul(out=dd, in0=d, in1=d)

            prm2 = cpool.tile([128, 8], F32)
            cd = prm2[:, 0:1]
            lndd = prm2[:, 1:2]
            dk3 = prm2[:, 2:3]
            nc.vector.tensor_mul(out=cd, in0=c, in1=d)

            # diag stationaries first -- they gate the PE pipeline start
            diag_d = cpool.tile([128, 128], BF16)
            diag_A = cpool.tile([128, 128], BF16)
            diag_c = cpool.tile([128, 128], BF16)
            diag_cd = cpool.tile([128, 128], BF16)
            nc.vector.tensor_scalar_mul(diag_d[:], I01, d)
            nc.vector.tensor_scalar_mul(diag_A[:], I01, A)
            nc.vector.tensor_scalar_mul(diag_c[:], I01, c)
            nc.vector.tensor_scalar_mul(diag_cd[:], I01, cd)

            dks2 = cpool.tile([128, 13], F32)

            def emit_dks2():
                nc.vector.tensor_scalar_mul(lndd, lnd, 2.0)
                nc.scalar.copy(out=dks2[:, 0:1], in_=dd)
                for j in range(1, 13):
                    nc.vector.tensor_mul(
                        out=dks2[:, j : j + 1],
                        in0=dks2[:, j - 1 : j],
                        in1=dks2[:, j - 1 : j],
                    )
                nc.vector.tensor_mul(out=dk3, in0=dks2[:, 11:12], in1=dks2[:, 12:13])

            # ---- G2: tile-interleaved geometric table over tiles 0-1:
            # G2[:, 1024e + j] = d^e * dd^j;  [2048:4096] = that * dd^1024 ----
            ramp = cpool.tile([128, 1024], F32)
            nc.gpsimd.iota(
                out=ramp[:],
                pattern=[[1, 1024]],
                base=0,
                channel_multiplier=0,
                allow_small_or_imprecise_dtypes=True,
            )
            G2 = bigpool.tile([128, 4096], BF16)
            g2_steps = [
                lambda: nc.scalar.activation(
                    G2[:, 0:1024], ramp[:], ACTF.Exp, scale=lndd
                ),
                lambda: nc.scalar.activation(
                    G2[:, 1024:2048], G2[:, 0:1024], ACTF.Copy, scale=d
                ),
                lambda: nc.scalar.activation(
                    G2[:, 2048:4096], G2[:, 0:2048], ACTF.Copy, scale=dks2[:, 10:11]
                ),
            ]

            # tile-interleaved output buffers: A = tiles 0-3, B = tiles 4-7
            ombufA = bigpool.tile([128, NK], BF16)
            ombufB = bigpool.tile([128, NK], BF16)

            def ombuf(t):
                return (ombufA, 2 * W * t) if t < 4 else (ombufB, 2 * W * (t - 4))

            zcol = cpool.tile([128, 1], BF16)
            nc.vector.memset(zcol[:, :], 0.0)

            identb = cpool.tile([128, 128], BF16)
            nc.scalar.copy(out=identb[:, :], in_=I01)

            def junk_mms(ut, pe, n):
                # HAM filler: overwritten by the next start=True matmul into
                # the same bank; keeps the PE activity window busy.
                for i in range(n):
                    nc.tensor.matmul(
                        ut[:, (i % 2) * MM : (i % 2) * MM + MM],
                        diag_A[:],
                        pe[:, 0:MM],
                        start=True,
                        stop=True,
                        skip_group_check=True,
                    )

            def u_tile(t, warm=0):
                lo = t * W
                pe = pslice(0, lo, W)
                po = pslice(1, lo, W)
                ut = psu.tile([128, W], F32, tag="u")
                junk_mms(ut, pe, warm)
                for q in range(W // MM):
                    nc.tensor.matmul(
                        ut[:, q * MM : (q + 1) * MM],
                        diag_d[:],
                        pe[:, q * MM : (q + 1) * MM],
                        start=True,
                        stop=False,
                    )
                for q in range(W // MM):
                    nc.tensor.matmul(
                        ut[:, q * MM : (q + 1) * MM],
                        identb[:],
                        po[:, q * MM : (q + 1) * MM],
                        start=False,
                        stop=True,
                    )
                return ut

            u_tiles = {0: u_tile(0, warm=0)}

            # ---- main loop ----
            prev_w = None
            for t in range(NTILES):
                lo = t * W
                ups = u_tiles.pop(t)
                if t + 1 < NTILES:
                    u_tiles[t + 1] = u_tile(t + 1)

                wt = wpool.tile([128, W + 1], BF16, tag="w")
                init = zcol[:, 0:1] if prev_w is None else prev_w[:, W : W + 1]
                nc.vector.tensor_tensor_scan(
                    out=wt[:, 1 : W + 1],
                    data0=dd.broadcast_to([128, W]),
                    data1=ups[:],
                    initial=init,
                    op0=ALU.mult,
                    op1=ALU.add,
                )
                # boundary copy off the scan chain: combine's c-run needs it
                # only ~1us after the scan completes
                nc.vector.tensor_copy(wt[:, 0:1], init)

                pe = pslice(0, lo, W)
                po = pslice(1, lo, W)
                ome = pse.tile([128, W], F32, tag="ome")
                omo = pso.tile([128, W], F32, tag="omo")
                # e-group first (A, c) so its drain fires mid-combine
                for q in range(W // MM):
                    nc.tensor.matmul(
                        ome[:, q * MM : (q + 1) * MM],
                        diag_A[:],
                        pe[:, q * MM : (q + 1) * MM],
                        start=True,
                        stop=False,
                    )
                for q in range(W // MM):
                    nc.tensor.matmul(
                        ome[:, q * MM : (q + 1) * MM],
                        diag_c[:],
                        wt[:, q * MM : q * MM + MM],
                        start=False,
                        stop=True,
                    )
                for q in range(W // MM):
                    nc.tensor.matmul(
                        omo[:, q * MM : (q + 1) * MM],
                        diag_A[:],
                        po[:, q * MM : (q + 1) * MM],
                        start=True,
                        stop=False,
                    )
                for q in range(W // MM):
                    nc.tensor.matmul(
                        omo[:, q * MM : (q + 1) * MM],
                        diag_c[:],
                        pe[:, q * MM : (q + 1) * MM],
                        start=False,
                        stop=False,
                    )
                for q in range(W // MM):
                    nc.tensor.matmul(
                        omo[:, q * MM : (q + 1) * MM],
                        diag_cd[:],
                        wt[:, q * MM : q * MM + MM],
                        start=False,
                        stop=True,
                    )

                ob, og = ombuf(t)
                nc.scalar.copy(out=ob[:, og : og + W], in_=ome[:])
                nc.scalar.copy(out=ob[:, og + W : og + 2 * W], in_=omo[:])

                if t == 3:
                    nc.gpsimd.dma_start(
                        out=out_ext[0:64, 0:8192], in_=ombufA[0:64, :]
                    )
                elif t == 7:
                    nc.gpsimd.dma_start(
                        out=out_ext[0:64, 8192:16384], in_=ombufB[0:64, :]
                    )
                if t == 0:
                    emit_dks2()
                elif t in (1, 2, 3):
                    g2_steps[t - 1]()
                prev_w = wt

            # ---- tail: fix up half 2 (rows 64:128) ----
            v1e = cpool.tile([128, 1], BF16)
            nc.sync.dma_start(out=v1e[64:128, :], in_=prev_w[0:64, W : W + 1])
            qs = cpool.tile([128, 4], F32)
            nc.vector.tensor_mul(
                out=qs[64:128, 0:1], in0=prm[64:128, 14:15], in1=v1e[64:128, :]
            )
            for ci, dcol in ((1, dks2[64:128, 11:12]), (2, dks2[64:128, 12:13]),
                             (3, prm2[64:128, 2:3])):
                nc.vector.tensor_mul(
                    out=qs[64:128, ci : ci + 1], in0=qs[64:128, 0:1], in1=dcol
                )

            # 4 chunks of 4096 over the tile-interleaved x axis
            for ci in range(4):
                xlo = 4096 * ci
                ob = ombufA if ci < 2 else ombufB
                og = xlo if ci < 2 else xlo - 8192
                fix = fxpool.tile([128, 4096], BF16, tag="fix")
                stage = stpool.tile([128, 4096], BF16, tag="stage")
                nc.vector.tensor_scalar_mul(
                    fix[64:128, :], G2[64:128, :], qs[64:128, ci : ci + 1]
                )
                nc.vector.tensor_add(
                    out=stage[64:128, :],
                    in0=fix[64:128, :],
                    in1=ob[64:128, og : og + 4096],
                )
                eng = nc.sync if ci % 2 == 0 else nc.gpsimd
                eng.dma_start(
                    out=out_ext[64:128, xlo : xlo + 4096],
                    in_=stage[64:128, :],
                )

    return nc


def make_nc():
    nc = bacc.Bacc(None)
    build(nc)
    nc.finalize()
    return nc


def _stage_p(p_core):
    # [64, 32768] f32 -> [128, 16384] bf16: q=h*64+b, x=e*8192+k
    x = np.asarray(p_core, dtype=BF).reshape(64, 2, NK, 2)
    return np.ascontiguousarray(x.transpose(1, 0, 3, 2).reshape(128, 2 * NK))


def _stage_hr(hr_core):
    # [64, 3] f32 -> [128, 160] f32: cols 0-2 params (rows duplicated
    # across halves), cols 32-159 a 0/1 identity matrix
    out = np.zeros((128, 160), dtype=np.float32)
    out[0:64, 0:3] = hr_core
    out[64:128, 0:3] = hr_core
    out[:, 32:160] = np.eye(128, dtype=np.float32)
    return out


def _unstage_out(o_core):
    # [128, 16384] bf16 tile-interleaved -> [64, 32768] f32
    x = np.asarray(o_core).reshape(2, 64, NTILES, 2, W)  # (h, b, t, e, j)
    x = x.transpose(1, 0, 2, 4, 3)  # (b, h, t, j, e)
    return np.ascontiguousarray(x.reshape(64, NT)).astype(np.float32)


def run(inputs, trace=False):
    nc = make_nc()
    p = np.asarray(inputs["p"], dtype=np.float32)
    hr = np.asarray(inputs["h_raw"], dtype=np.float32)
    in_maps = []
    for i in range(NCORES):
        sl = slice(i * BLOC, (i + 1) * BLOC)
        in_maps.append({"p": _stage_p(p[sl]), "h_raw": _stage_hr(hr[sl])})
    res = run_bass_kernel_spmd(nc, in_maps, core_ids=list(range(NCORES)), trace=trace)
    out = np.concatenate(
        [_unstage_out(res.results[i]["out"]) for i in range(NCORES)], axis=0
    )
    return out, res


def kernel(h, t, p, h_raw):
    out, _ = run({"p": p, "h_raw": h_raw})
    return out
